# revision 4
# baseline (speedup 1.0000x reference)
"""Multi-head attention (B=2, S=2048, H=1024, 16 heads x 64) on 8 trn2 cores.

Sharding: data-parallel over batch (2) x tensor-parallel over heads (4 groups
of 4 heads). Core c handles batch c//4, head-group c%4 (wq/wk/wv columns
[256*g, 256*g+256)). Host slices inputs per core (shipping q/k/v pre-cast to
bf16 - the kernel's chosen compute precision - and pre-transposed to the
[H, S] layout the SBUF tiles use) and concatenates the per-core head-slice
outputs.

Per-core schedule (bf16 matmul operands, fp32 PSUM accumulation):
  ACT (exp over the 4*S*S scores) is the pacing engine at ~18.3us/segment;
  the PE runs scores (row-packed K=64 head pairs, concurrent via
  tile_position (0,0)/(64,0)), PV (65-wide stationary [V|ones] so the
  softmax denominator rides along), projections and V transposes at
  ~17us/segment underneath it.

  - prefix: one 3D-AP DMA per (tensor, 512-col chunk) (18 input triggers
    total; ~600ns/trigger on the sync/scalar queues is why they are
    batched), PE warm-up matmuls during the DMA wait (HAM un-throttle),
    then just enough projections (k nt0-1, q nt0, v nt0-1) to start the
    exp stream.
  - steady state: EGRP=2 score units share a [128,1024] PSUM tile per exp
    call; remaining projection work drip-feeds into PE slack via deadline
    fillers.
  - finalize: PSUM [65,512] out'^T tiles are copied to bf16 SBUF and
    transposed by the DMA xbar ([80,128] chunks -> [128,80]) instead of
    the PE; reciprocal of the denominator column + per-row scale write the
    staged [q,256] f32 output tiles, DMA'd out per q-tile.

The softmax mask of the reference is a mathematical no-op (it broadcasts
over the key axis, shifting every logit of a row equally), so it is ignored.
"""

import numpy as np

B, S, H = 2, 2048, 1024
NH, D = 16, 64            # heads, head_dim
CORES = 8
GROUP_COLS = 256          # 4 heads per core
SCALE = 1.0 / 32.0        # 1/sqrt(H)
EGRP = 2                  # score units (512 q cols) per exp batch

_CACHE = {}


def _build():
    import concourse.bacc as bacc
    import concourse.tile as tile
    import concourse.mybir as mybir
    from concourse.masks import make_identity
    from contextlib import ExitStack

    F32 = mybir.dt.float32
    BF16 = mybir.dt.bfloat16
    EXP = mybir.ActivationFunctionType.Exp

    nc = bacc.Bacc("TRN2", target_bir_lowering=False, debug=False,
                   num_devices=CORES)

    q_d = nc.dram_tensor("q", [H, S], BF16, kind="ExternalInput").ap()
    k_d = nc.dram_tensor("k", [H, S], BF16, kind="ExternalInput").ap()
    v_d = nc.dram_tensor("v", [H, S], BF16, kind="ExternalInput").ap()
    w_d = {x: nc.dram_tensor("w" + x, [H, GROUP_COLS], BF16,
                             kind="ExternalInput").ap() for x in "qkv"}
    b_d = {x: nc.dram_tensor("b" + x, [GROUP_COLS, 1], F32,
                             kind="ExternalInput").ap() for x in "qkv"}
    out_d = nc.dram_tensor("out", [S, GROUP_COLS], F32,
                           kind="ExternalOutput").ap()
    x_d = {"q": q_d, "k": k_d, "v": v_d}

    NS = S // 128          # 16 key tiles
    NK = H // 128          # 8 contraction tiles over H
    NQ = S // 512          # 4 q-tiles of 512
    NM = 2                 # head-pairs per core

    with tile.TileContext(nc) as tc, ExitStack() as es:
        const = es.enter_context(tc.tile_pool(name="const", bufs=1))
        wpool = es.enter_context(tc.tile_pool(name="w", bufs=1))
        xT = es.enter_context(tc.tile_pool(name="xT", bufs=1))
        proj = es.enter_context(tc.tile_pool(name="proj", bufs=1))
        vchunkp = es.enter_context(tc.tile_pool(name="vchunk", bufs=2))
        vhp = es.enter_context(tc.tile_pool(name="vh", bufs=1))
        pexpp = es.enter_context(tc.tile_pool(name="pexp", bufs=8))
        sbap = es.enter_context(tc.tile_pool(name="sba", bufs=4))
        tsbp = es.enter_context(tc.tile_pool(name="tsb", bufs=12))
        stagep = es.enter_context(tc.tile_pool(name="stage", bufs=16))
        recp = es.enter_context(tc.tile_pool(name="rec", bufs=8))
        # PSUM: sc = [128,1024] x2 = 4 banks; pa (proj acc / V transposes /
        # warmup) = 2 banks; pva/pvb = 2 banks.
        ps_sc = es.enter_context(tc.tile_pool(name="ps_sc", bufs=2, space="PSUM"))
        ps_pa = es.enter_context(tc.tile_pool(name="ps_pa", bufs=2, space="PSUM"))
        ps_pv = es.enter_context(tc.tile_pool(name="ps_pv", bufs=1, space="PSUM"))

        identb = const.tile([128, 128], BF16, tag="identb")
        make_identity(nc, identb[:])

        # ---- PE warm-up: keep the array busy during the DMA prefix so the
        # HAM clock-gate releases (K=8/8) before real projections start.
        warm = const.tile([128, 512], BF16, tag="warm")
        nc.vector.memset(warm[:], 0.0)
        wps = ps_pa.tile([128, 512], F32, tag="pa", name="warmps")
        for _ in range(12):
            nc.tensor.matmul(wps[:], warm[:, 0:128], warm[:],
                             start=True, stop=True)

        bias_t = {}
        for x in "qkv":
            bt = const.tile([128, NM], F32, tag=f"b{x}")
            nc.sync.dma_start(
                out=bt[:], in_=b_d[x].rearrange("(m p) o -> p m o", p=128)
                .rearrange("p m o -> p (m o)"))
            for m in range(NM):
                bias_t[(x, m)] = bt[:, m:m + 1]

        # weights: one 3D-AP DMA per tensor
        wbf = {}
        for x in "kqv":
            wb = wpool.tile([128, NK, GROUP_COLS], BF16, tag=f"wb{x}",
                            name=f"wb_{x}")
            nc.sync.dma_start(
                out=wb[:], in_=w_d[x].rearrange("(kb p) c -> p kb c", p=128))
            for kb in range(NK):
                wbf[(x, kb)] = wb[:, kb, :]

        # x tiles: one [128, NK, S] tile per tensor; one 3D-AP DMA per
        # (tensor, 512-col chunk), ordered for earliest exp start. The
        # critical prefix chunks alternate sync/scalar queues.
        xTt = {}
        for x in "kqv":
            xTt[x] = xT.tile([128, NK, S], BF16, tag=f"xt{x}", name=f"xT_{x}")

        def dma_x(x, nt, eng):
            eng.dma_start(
                out=xTt[x][:, :, 512 * nt:512 * nt + 512],
                in_=x_d[x].rearrange("(kb p) c -> p kb c", p=128)
                [:, :, 512 * nt:512 * nt + 512])

        dma_x("k", 0, nc.scalar)
        dma_x("q", 0, nc.sync)
        dma_x("v", 0, nc.scalar)
        dma_x("k", 1, nc.sync)
        dma_x("v", 1, nc.scalar)
        dma_x("k", 2, nc.sync)
        dma_x("v", 2, nc.sync)
        dma_x("k", 3, nc.sync)
        dma_x("v", 3, nc.sync)
        dma_x("q", 1, nc.sync)
        dma_x("q", 2, nc.sync)
        dma_x("q", 3, nc.sync)

        def xslice(x, kb, lo, hi):
            return xTt[x][:, kb, lo:hi]

        # persistent projection outputs
        QT = [proj.tile([128, S], BF16, tag=f"qt{m}", name=f"QT{m}")
              for m in range(NM)]
        KT = [proj.tile([128, S], BF16, tag=f"kt{m}", name=f"KT{m}")
              for m in range(NM)]
        VH = [[vhp.tile([128, 129], BF16, tag=f"vh{m}_{s}", name=f"VH{m}_{s}")
               for s in range(NS)] for m in range(NM)]
        for m in range(NM):
            for s in range(NS):
                nc.vector.memset(VH[m][s][:, 64:65], 1.0)

        def proj_qk_nt(x, m, nt):
            acc = ps_pa.tile([128, 512], F32, tag="pa", name="acc")
            for kb in range(NK):
                nc.tensor.matmul(
                    acc[:], wbf[(x, kb)][:, 128 * m:128 * m + 128],
                    xslice(x, kb, 512 * nt, 512 * nt + 512),
                    start=(kb == 0), stop=(kb == NK - 1))
            dst = (QT if x == "q" else KT)[m][:, 512 * nt:512 * nt + 512]
            nc.vector.tensor_scalar_add(dst, acc[:], bias_t[(x, m)])

        def proj_v_nt(m, nt):
            acc = ps_pa.tile([128, 512], F32, tag="pa", name="acc")
            for kb in range(NK):
                nc.tensor.matmul(
                    acc[:], wbf[("v", kb)][:, 128 * m:128 * m + 128],
                    xslice("v", kb, 512 * nt, 512 * nt + 512),
                    start=(kb == 0), stop=(kb == NK - 1))
            vchunk = vchunkp.tile([128, 512], BF16, tag="vchunk", name="vchunk")
            nc.vector.tensor_scalar_add(vchunk[:], acc[:], bias_t[("v", m)])
            for i in range(4):
                s = 4 * nt + i
                trp = ps_pa.tile([128, 128], BF16, tag="pa", name="trv")
                nc.tensor.transpose(trp[:], vchunk[:, 128 * i:128 * i + 128],
                                    identb[:])
                vt = VH[m][s]
                nc.vector.tensor_copy(vt[:, 0:64], trp[:, 0:64])
                nc.vector.tensor_copy(vt[:, 65:129], trp[:, 64:128])

        stages = {}
        for qt in range(NQ):
            stages[qt] = [stagep.tile([128, GROUP_COLS], F32, tag="stage",
                                      name=f"stage{qt}_{i}") for i in range(4)]

        # ---- pre-work: just enough to start the exp stream ----
        proj_qk_nt("k", 0, 0)
        proj_qk_nt("k", 0, 1)
        proj_qk_nt("q", 0, 0)
        proj_v_nt(0, 0)
        proj_v_nt(0, 1)

        # ---- attention pipeline with deadline-driven PE fillers ----
        units = [(kt, a) for kt in range(NS) for a in (0, 1)]
        grps = [units[i:i + EGRP] for i in range(0, len(units), EGRP)]
        NG = len(grps)

        # m-major segment order
        segs = [{"qt": qt, "m": m, "pva": None, "pvb": None, "idx": 4 * m + qt}
                for m in range(NM) for qt in range(NQ)]

        # fillers: (deadline (seg_idx, gi) = emit before that slot's pv, fn)
        fq = [
            ((0, 5), lambda: proj_qk_nt("k", 0, 2)),
            ((0, 7), lambda: proj_v_nt(0, 2)),
            ((0, 9), lambda: proj_qk_nt("k", 0, 3)),
            ((0, 11), lambda: proj_v_nt(0, 3)),
            ((0, 14), lambda: proj_qk_nt("q", 0, 1)),    # QT[0] for seg 1
            ((1, 3), lambda: proj_qk_nt("k", 1, 0)),
            ((1, 7), lambda: proj_qk_nt("k", 1, 1)),
            ((1, 14), lambda: proj_qk_nt("q", 0, 2)),    # QT[0] for seg 2
            ((2, 3), lambda: proj_qk_nt("k", 1, 2)),
            ((2, 7), lambda: proj_qk_nt("k", 1, 3)),
            ((2, 14), lambda: proj_qk_nt("q", 0, 3)),    # QT[0] for seg 3
            ((3, 3), lambda: proj_v_nt(1, 0)),           # VH[1][kt 0..3]
            ((3, 14), lambda: proj_qk_nt("q", 1, 0)),    # QT[1] for seg 4
            ((4, 3), lambda: proj_v_nt(1, 1)),
            ((4, 7), lambda: proj_v_nt(1, 2)),
            ((4, 11), lambda: proj_v_nt(1, 3)),
            ((4, 14), lambda: proj_qk_nt("q", 1, 1)),    # QT[1] for seg 5
            ((5, 14), lambda: proj_qk_nt("q", 1, 2)),
            ((6, 14), lambda: proj_qk_nt("q", 1, 3)),
        ]
        fq.sort(key=lambda fd: fd[0])

        def pump(upto):
            while fq and fq[0][0] <= upto:
                fq.pop(0)[1]()

        def emit_scores(seg, g):
            qt, m = seg["qt"], seg["m"]
            stt = ps_sc.tile([128, 1024], F32, tag="sc", name="stt")
            for u, (kt, a) in enumerate(g):
                p0 = 64 * a
                nc.tensor.matmul(
                    stt[:, 512 * u:512 * u + 512],
                    KT[m][p0:p0 + 64, 128 * kt:128 * kt + 128],
                    QT[m][p0:p0 + 64, 512 * qt:512 * qt + 512],
                    start=True, stop=True, tile_position=(p0, 0))
            pe = pexpp.tile([128, 1024], BF16, tag="pexp", name="pexp")
            n = 512 * len(g)
            nc.scalar.activation(pe[:, 0:n], stt[:, 0:n], EXP, scale=SCALE)
            return pe

        def emit_pv(seg, g, pe):
            m = seg["m"]
            if seg["pva"] is None:
                seg["pva"] = ps_pv.tile([65, 512], F32, tag="pva", name="pva")
                seg["pvb"] = ps_pv.tile([65, 512], F32, tag="pvb", name="pvb")
            for u, (kt, a) in enumerate(g):
                pv = seg["pva"] if a == 0 else seg["pvb"]
                lo = 64 * a
                nc.tensor.matmul(pv[:], VH[m][kt][:, lo:lo + 65],
                                 pe[:, 512 * u:512 * u + 512],
                                 start=(kt == 0), stop=(kt == NS - 1))

        # finalize: PSUM->SBUF bf16 copies run immediately (freeing the PSUM
        # banks) along with the xbar DMA transposes; the reciprocal/scale
        # steps become fillers spread over the following segment's slack.
        def fin_item(seg, tsb, sub, a):
            # pva cols were [A(64)|ones] -> denom at col 64, dims at 0:64;
            # pvb cols were [ones|B(64)] -> denom at col 0, dims at 1:65.
            qt, m = seg["qt"], seg["m"]
            stage = stages[qt]
            dcol = 64 if a == 0 else 0
            lo, hi = (0, 64) if a == 0 else (1, 65)
            r = recp.tile([128, 1], F32, tag="rec", name="r")
            nc.vector.reciprocal(r[:], tsb[:, dcol:dcol + 1])
            nc.vector.tensor_scalar_mul(
                stage[sub][:, 128 * m + 64 * a:128 * m + 64 * a + 64],
                tsb[:, lo:hi], r[:, 0:1])
            seg["fin_done"] = seg.get("fin_done", 0) + 1
            if seg["fin_done"] == 8 and m == NM - 1:
                for s2 in range(4):
                    nc.sync.dma_start(
                        out=out_d[512 * qt + 128 * s2:512 * qt + 128 * s2 + 128, :],
                        in_=stage[s2][:])

        flat = [(seg, gi) for seg in segs for gi in range(NG)]
        pending = emit_scores(flat[0][0], grps[flat[0][1]])
        for j, (seg, gi) in enumerate(flat):
            if j + 1 < len(flat):
                nseg, ngi = flat[j + 1]
                nxt = emit_scores(nseg, grps[ngi])
            else:
                nxt = None
            pump((seg["idx"], gi))
            emit_pv(seg, grps[gi], pending)
            if gi == NG - 1:
                nidx = seg["idx"] + 1
                for a in (0, 1):
                    pv = seg["pva"] if a == 0 else seg["pvb"]
                    sb = sbap.tile([80, 512], BF16, tag="sba", name="sb")
                    # partition starts must be 32-aligned: memset [64:80)
                    # first, the copy then overwrites row 64 with the denom
                    nc.vector.memset(sb[64:80, :], 0.0)
                    nc.vector.tensor_copy(sb[0:65, :], pv[:])
                    for sub in range(4):
                        tsb = tsbp.tile([128, 80], BF16, tag="tsb", name="tsb")
                        nc.sync.dma_start_transpose(
                            tsb[:], sb[0:80, 128 * sub:128 * sub + 128])
                        fq.append(((nidx, 1 + 2 * sub + a),
                                   (lambda s_=seg, t_=tsb, su_=sub, a_=a:
                                    fin_item(s_, t_, su_, a_))))
                fq.sort(key=lambda fd: fd[0])
            pending = nxt
        pump((99, 99))    # drain remaining fillers (last segment's finalize)

    nc.compile()
    return nc


def _get_nc():
    if "nc" not in _CACHE:
        _CACHE["nc"] = _build()
    return _CACHE["nc"]


def _run(inputs, trace=False, tmpdir=None):
    import ml_dtypes
    from concourse.bass_utils import run_bass_kernel_spmd

    nc = _get_nc()
    q, k, v = inputs["q"], inputs["k"], inputs["v"]
    wq, wk, wv = inputs["wq"], inputs["wk"], inputs["wv"]
    bq, bk, bv = inputs["bq"], inputs["bk"], inputs["bv"]

    def f32(a):
        return np.ascontiguousarray(np.asarray(a), dtype=np.float32)

    def bf16w(a):
        return np.ascontiguousarray(
            np.asarray(a, dtype=np.float32).astype(ml_dtypes.bfloat16))

    def bf16_t(a):
        # pre-cast to the kernel's bf16 compute precision and pre-transpose
        # to the [H, S] layout its SBUF tiles use
        return np.ascontiguousarray(
            np.asarray(a, dtype=np.float32).astype(ml_dtypes.bfloat16).T)

    in_maps = []
    for c in range(CORES):
        b, g = divmod(c, CORES // B)
        sel = slice(GROUP_COLS * g, GROUP_COLS * g + GROUP_COLS)
        in_maps.append({
            "q": bf16_t(q[b]), "k": bf16_t(k[b]), "v": bf16_t(v[b]),
            "wq": bf16w(wq[:, sel]), "wk": bf16w(wk[:, sel]),
            "wv": bf16w(wv[:, sel]),
            "bq": f32(bq[sel]).reshape(GROUP_COLS, 1),
            "bk": f32(bk[sel]).reshape(GROUP_COLS, 1),
            "bv": f32(bv[sel]).reshape(GROUP_COLS, 1),
        })

    res = run_bass_kernel_spmd(nc, in_maps, list(range(CORES)),
                               trace=trace, tmpdir=tmpdir)
    out = np.empty((B, S, H), dtype=np.float32)
    for c in range(CORES):
        b, g = divmod(c, CORES // B)
        out[b, :, GROUP_COLS * g:GROUP_COLS * g + GROUP_COLS] = \
            res.results[c]["out"]
    return out, res


def kernel(**inputs):
    out, _ = _run(inputs, trace=False)
    return out


# revision 11
# speedup vs baseline: 1.1871x; 1.1871x over previous
"""Multi-head attention (B=2, S=2048, H=1024, 16 heads x 64) on 8 trn2 cores.

Sharding: data-parallel over batch (2) x tensor-parallel over heads (4 groups
of 4 heads). Core c handles batch c//4, head-group c%4 (wq/wk/wv columns
[256*g, 256*g+256)). Host slices inputs per core (shipping q/k/v pre-cast to
bf16 - the kernel's chosen compute precision - and pre-transposed to the
[H, S] layout the SBUF tiles use) and concatenates the per-core head-slice
outputs.

Per-core schedule (bf16 matmul operands, fp32 PSUM accumulation):
  ACT (exp over the 4*S*S scores) is the pacing engine at ~18.3us/segment;
  the PE runs scores (row-packed K=64 head pairs, concurrent via
  tile_position (0,0)/(64,0)), PV (65-wide stationary [V|ones] so the
  softmax denominator rides along), projections and V transposes at
  ~17us/segment underneath it.

  - prefix: one 3D-AP DMA per (tensor, 512-col chunk) (18 input triggers
    total; ~600ns/trigger on the sync/scalar queues is why they are
    batched), PE warm-up matmuls during the DMA wait (HAM un-throttle),
    then just enough projections (k nt0-1, q nt0, v nt0-1) to start the
    exp stream.
  - steady state: EGRP=2 score units share a [128,1024] PSUM tile per exp
    call; remaining projection work drip-feeds into PE slack via deadline
    fillers.
  - finalize: PSUM [65,512] out'^T tiles are copied to bf16 SBUF and
    transposed by the DMA xbar ([80,128] chunks -> [128,80]) instead of
    the PE; reciprocal of the denominator column + per-row scale write the
    staged [q,256] f32 output tiles, DMA'd out per q-tile.

The softmax mask of the reference is a mathematical no-op (it broadcasts
over the key axis, shifting every logit of a row equally), so it is ignored.
"""

import numpy as np

B, S, H = 2, 2048, 1024
NH, D = 16, 64            # heads, head_dim
CORES = 8
GROUP_COLS = 256          # 4 heads per core
SCALE = 1.0 / 32.0        # 1/sqrt(H)
EGRP = 2                  # score units (512 q cols) per exp batch

_CACHE = {}


def _build():
    import concourse.bacc as bacc
    import concourse.tile as tile
    import concourse.mybir as mybir
    from concourse.masks import make_identity
    from contextlib import ExitStack

    F32 = mybir.dt.float32
    BF16 = mybir.dt.bfloat16
    EXP = mybir.ActivationFunctionType.Exp

    nc = bacc.Bacc("TRN2", target_bir_lowering=False, debug=False,
                   num_devices=CORES)

    q_d = nc.dram_tensor("q", [H, S], BF16, kind="ExternalInput").ap()
    k_d = nc.dram_tensor("k", [H, S], BF16, kind="ExternalInput").ap()
    v_d = nc.dram_tensor("v", [H, S], BF16, kind="ExternalInput").ap()
    w_d = {x: nc.dram_tensor("w" + x, [H, GROUP_COLS], BF16,
                             kind="ExternalInput").ap() for x in "qkv"}
    b_d = {x: nc.dram_tensor("b" + x, [GROUP_COLS, 1], F32,
                             kind="ExternalInput").ap() for x in "qkv"}
    out_d = nc.dram_tensor("out", [S, GROUP_COLS], F32,
                           kind="ExternalOutput").ap()
    x_d = {"q": q_d, "k": k_d, "v": v_d}

    NS = S // 128          # 16 key tiles
    NK = H // 128          # 8 contraction tiles over H
    NQ = S // 512          # 4 q-tiles of 512
    NM = 2                 # head-pairs per core

    with tile.TileContext(nc) as tc, ExitStack() as es:
        const = es.enter_context(tc.tile_pool(name="const", bufs=1))
        wpool = es.enter_context(tc.tile_pool(name="w", bufs=1))
        xT = es.enter_context(tc.tile_pool(name="xT", bufs=1))
        proj = es.enter_context(tc.tile_pool(name="proj", bufs=1))
        vchunkp = es.enter_context(tc.tile_pool(name="vchunk", bufs=2))
        vhp = es.enter_context(tc.tile_pool(name="vh", bufs=1))
        pexpp = es.enter_context(tc.tile_pool(name="pexp", bufs=8))
        sbap = es.enter_context(tc.tile_pool(name="sba", bufs=4))
        tsbp = es.enter_context(tc.tile_pool(name="tsb", bufs=8))
        stagep = es.enter_context(tc.tile_pool(name="stage", bufs=16))
        recp = es.enter_context(tc.tile_pool(name="rec", bufs=8))
        # PSUM: sc = [128,1024] x2 = 4 banks; pa (proj acc / V transposes /
        # warmup) = 2 banks; pva/pvb = 2 banks.
        ps_sc = es.enter_context(tc.tile_pool(name="ps_sc", bufs=2, space="PSUM"))
        ps_pa = es.enter_context(tc.tile_pool(name="ps_pa", bufs=2, space="PSUM"))
        ps_pv = es.enter_context(tc.tile_pool(name="ps_pv", bufs=1, space="PSUM"))

        ident = const.tile([128, 128], F32, tag="ident")
        make_identity(nc, ident[:])
        identb = const.tile([128, 128], BF16, tag="identb")
        make_identity(nc, identb[:])

        # ---- PE warm-up: keep the array busy during the DMA prefix so the
        # HAM clock-gate releases (K=8/8) before real projections start.
        warm = const.tile([128, 512], BF16, tag="warm")
        nc.vector.memset(warm[:], 0.0)
        wps = ps_pa.tile([128, 512], F32, tag="pa", name="warmps")
        for _ in range(12):
            nc.tensor.matmul(wps[:], warm[:, 0:128], warm[:],
                             start=True, stop=True)

        bias_t = {}
        for x in "qkv":
            bt = const.tile([128, NM], F32, tag=f"b{x}")
            nc.sync.dma_start(
                out=bt[:], in_=b_d[x].rearrange("(m p) o -> p m o", p=128)
                .rearrange("p m o -> p (m o)"))
            for m in range(NM):
                bias_t[(x, m)] = bt[:, m:m + 1]

        # weights: one 3D-AP DMA per tensor
        wbf = {}
        for x in "kqv":
            wb = wpool.tile([128, NK, GROUP_COLS], BF16, tag=f"wb{x}",
                            name=f"wb_{x}")
            nc.sync.dma_start(
                out=wb[:], in_=w_d[x].rearrange("(kb p) c -> p kb c", p=128))
            for kb in range(NK):
                wbf[(x, kb)] = wb[:, kb, :]

        # x tiles: one [128, NK, S] tile per tensor. Critical prefix chunks
        # are split into per-kb-slice DMAs across many queues (a single
        # [128,8,512] 1MB DMA runs at ~75GB/s on one queue = 13us, gating
        # the exp-stream start); late chunks use one 3D-AP trigger each
        # (trigger instructions cost ~600ns on the issuing engine).
        xTt = {}
        for x in "kqv":
            xTt[x] = xT.tile([128, NK, S], BF16, tag=f"xt{x}", name=f"xT_{x}")

        def dma_x(x, nt, eng, kb0=0, kb1=NK):
            eng.dma_start(
                out=xTt[x][:, kb0:kb1, 512 * nt:512 * nt + 512],
                in_=x_d[x].rearrange("(kb p) c -> p kb c", p=128)
                [:, kb0:kb1, 512 * nt:512 * nt + 512])

        engs = (nc.sync, nc.scalar)
        for kb in range(NK):                      # k nt0: 8 queues
            dma_x("k", 0, engs[kb % 2], kb, kb + 1)
        for i in range(4):                        # q nt0: 4 x 2-kb slices
            dma_x("q", 0, engs[i % 2], 2 * i, 2 * i + 2)
        for i in range(4):                        # v nt0
            dma_x("v", 0, engs[i % 2], 2 * i, 2 * i + 2)
        for i in range(2):                        # k nt1, v nt1
            dma_x("k", 1, engs[i], 4 * i, 4 * i + 4)
        for i in range(2):
            dma_x("v", 1, engs[i], 4 * i, 4 * i + 4)
        # late chunks all on sync: a queue-slot wait on the scalar engine
        # would block the exp stream behind it
        dma_x("k", 2, nc.sync)
        dma_x("v", 2, nc.sync)
        dma_x("k", 3, nc.sync)
        dma_x("v", 3, nc.sync)
        dma_x("q", 1, nc.sync)
        dma_x("q", 2, nc.sync)
        dma_x("q", 3, nc.sync)

        def xslice(x, kb, lo, hi):
            return xTt[x][:, kb, lo:hi]

        # persistent projection outputs
        QT = [proj.tile([128, S], BF16, tag=f"qt{m}", name=f"QT{m}")
              for m in range(NM)]
        KT = [proj.tile([128, S], BF16, tag=f"kt{m}", name=f"KT{m}")
              for m in range(NM)]
        VH = [[vhp.tile([128, 129], BF16, tag=f"vh{m}_{s}", name=f"VH{m}_{s}")
               for s in range(NS)] for m in range(NM)]
        for m in range(NM):
            for s in range(NS):
                nc.vector.memset(VH[m][s][:, 64:65], 1.0)

        def proj_qk_nt(x, m, nt):
            acc = ps_pa.tile([128, 512], F32, tag="pa", name="acc")
            for kb in range(NK):
                nc.tensor.matmul(
                    acc[:], wbf[(x, kb)][:, 128 * m:128 * m + 128],
                    xslice(x, kb, 512 * nt, 512 * nt + 512),
                    start=(kb == 0), stop=(kb == NK - 1))
            dst = (QT if x == "q" else KT)[m][:, 512 * nt:512 * nt + 512]
            nc.vector.tensor_scalar_add(dst, acc[:], bias_t[(x, m)])

        def proj_v_nt(m, nt):
            acc = ps_pa.tile([128, 512], F32, tag="pa", name="acc")
            for kb in range(NK):
                nc.tensor.matmul(
                    acc[:], wbf[("v", kb)][:, 128 * m:128 * m + 128],
                    xslice("v", kb, 512 * nt, 512 * nt + 512),
                    start=(kb == 0), stop=(kb == NK - 1))
            vchunk = vchunkp.tile([128, 512], BF16, tag="vchunk", name="vchunk")
            nc.vector.tensor_scalar_add(vchunk[:], acc[:], bias_t[("v", m)])
            for i in range(4):
                s = 4 * nt + i
                trp = ps_pa.tile([128, 128], BF16, tag="pa", name="trv")
                nc.tensor.transpose(trp[:], vchunk[:, 128 * i:128 * i + 128],
                                    identb[:])
                vt = VH[m][s]
                nc.vector.tensor_copy(vt[:, 0:64], trp[:, 0:64])
                nc.vector.tensor_copy(vt[:, 65:129], trp[:, 64:128])

        stages = {}
        for qt in range(NQ):
            stages[qt] = [stagep.tile([128, GROUP_COLS], F32, tag="stage",
                                      name=f"stage{qt}_{i}") for i in range(4)]

        # ---- pre-work: just enough to start the exp stream ----
        proj_qk_nt("k", 0, 0)
        proj_qk_nt("q", 0, 0)
        proj_v_nt(0, 0)
        proj_qk_nt("k", 0, 1)
        proj_v_nt(0, 1)

        # ---- attention pipeline with deadline-driven PE fillers ----
        units = [(kt, a) for kt in range(NS) for a in (0, 1)]
        grps = [units[i:i + EGRP] for i in range(0, len(units), EGRP)]
        NG = len(grps)

        # m-major segment order
        segs = [{"qt": qt, "m": m, "pva": None, "pvb": None, "idx": 4 * m + qt}
                for m in range(NM) for qt in range(NQ)]

        # fillers: (deadline (seg_idx, gi) = emit before that slot's pv, fn)
        fq = [
            ((0, 5), lambda: proj_qk_nt("k", 0, 2)),
            ((0, 7), lambda: proj_v_nt(0, 2)),
            ((0, 9), lambda: proj_qk_nt("k", 0, 3)),
            ((0, 11), lambda: proj_v_nt(0, 3)),
            ((0, 14), lambda: proj_qk_nt("q", 0, 1)),    # QT[0] for seg 1
            ((1, 3), lambda: proj_qk_nt("k", 1, 0)),
            ((1, 7), lambda: proj_qk_nt("k", 1, 1)),
            ((1, 14), lambda: proj_qk_nt("q", 0, 2)),    # QT[0] for seg 2
            ((2, 3), lambda: proj_qk_nt("k", 1, 2)),
            ((2, 7), lambda: proj_qk_nt("k", 1, 3)),
            ((2, 14), lambda: proj_qk_nt("q", 0, 3)),    # QT[0] for seg 3
            ((3, 3), lambda: proj_v_nt(1, 0)),           # VH[1][kt 0..3]
            ((3, 14), lambda: proj_qk_nt("q", 1, 0)),    # QT[1] for seg 4
            ((4, 3), lambda: proj_v_nt(1, 1)),
            ((4, 7), lambda: proj_v_nt(1, 2)),
            ((4, 11), lambda: proj_v_nt(1, 3)),
            ((4, 14), lambda: proj_qk_nt("q", 1, 1)),    # QT[1] for seg 5
            ((5, 14), lambda: proj_qk_nt("q", 1, 2)),
            ((6, 14), lambda: proj_qk_nt("q", 1, 3)),
        ]
        fq.sort(key=lambda fd: fd[0])

        def pump(upto):
            while fq and fq[0][0] <= upto:
                fq.pop(0)[1]()

        def emit_scores(seg, g):
            qt, m = seg["qt"], seg["m"]
            stt = ps_sc.tile([128, 1024], F32, tag="sc", name="stt")
            for u, (kt, a) in enumerate(g):
                p0 = 64 * a
                nc.tensor.matmul(
                    stt[:, 512 * u:512 * u + 512],
                    KT[m][p0:p0 + 64, 128 * kt:128 * kt + 128],
                    QT[m][p0:p0 + 64, 512 * qt:512 * qt + 512],
                    start=True, stop=True, tile_position=(p0, 0))
            pe = pexpp.tile([128, 1024], BF16, tag="pexp", name="pexp")
            n = 512 * len(g)
            nc.scalar.activation(pe[:, 0:n], stt[:, 0:n], EXP, scale=SCALE)
            return pe

        def emit_pv(seg, g, pe):
            m = seg["m"]
            if seg["pva"] is None:
                seg["pva"] = ps_pv.tile([65, 512], F32, tag="pva", name="pva")
                seg["pvb"] = ps_pv.tile([65, 512], F32, tag="pvb", name="pvb")
            for u, (kt, a) in enumerate(g):
                pv = seg["pva"] if a == 0 else seg["pvb"]
                lo = 64 * a
                nc.tensor.matmul(pv[:], VH[m][kt][:, lo:lo + 65],
                                 pe[:, 512 * u:512 * u + 512],
                                 start=(kt == 0), stop=(kt == NS - 1))

        # finalize: the pva/pvb->SBUF copies run immediately (freeing the
        # PSUM banks); the transpose/divide/stage steps become fillers
        # spread over the following segment's PE slack.
        # pva cols were [A(64)|ones] -> denom at row 64, dims at 0:64;
        # pvb cols were [ones|B(64)] -> denom at row 0, dims at 1:65.
        def fin_item(seg, sb, sub, a):
            qt, m = seg["qt"], seg["m"]
            stage = stages[qt]
            trp = ps_pa.tile([128, 128], F32, tag="pa", name="trf")
            nc.tensor.transpose(trp[:, 0:65],
                                sb[0:65, 128 * sub:128 * sub + 128],
                                ident[0:65, 0:65])
            # one fast copy releases the PSUM slot; divide from SBUF
            tsb = tsbp.tile([128, 65], F32, tag="tsb", name="tsb")
            nc.vector.tensor_copy(tsb[:], trp[:, 0:65])
            r = recp.tile([128, 1], F32, tag="rec", name="r")
            dcol = 64 if a == 0 else 0
            lo, hi = (0, 64) if a == 0 else (1, 65)
            nc.vector.reciprocal(r[:], tsb[:, dcol:dcol + 1])
            nc.vector.tensor_scalar_mul(
                stage[sub][:, 128 * m + 64 * a:128 * m + 64 * a + 64],
                tsb[:, lo:hi], r[:, 0:1])
            seg["fin_done"] = seg.get("fin_done", 0) + 1
            if seg["fin_done"] == 8 and m == NM - 1:
                for s2 in range(4):
                    nc.sync.dma_start(
                        out=out_d[512 * qt + 128 * s2:512 * qt + 128 * s2 + 128, :],
                        in_=stage[s2][:])

        flat = [(seg, gi) for seg in segs for gi in range(NG)]
        pending = emit_scores(flat[0][0], grps[flat[0][1]])
        for j, (seg, gi) in enumerate(flat):
            if j + 1 < len(flat):
                nseg, ngi = flat[j + 1]
                nxt = emit_scores(nseg, grps[ngi])
            else:
                nxt = None
            pump((seg["idx"], gi))
            emit_pv(seg, grps[gi], pending)
            if gi == NG - 1:
                nidx = seg["idx"] + 1
                for a in (0, 1):
                    pv = seg["pva"] if a == 0 else seg["pvb"]
                    sb = sbap.tile([65, 512], F32, tag="sba", name="sb")
                    nc.vector.tensor_copy(sb[:], pv[:])
                    for sub in range(4):
                        fq.append(((nidx, 1 + 2 * sub + a),
                                   (lambda s_=seg, sb_=sb, su_=sub, a_=a:
                                    fin_item(s_, sb_, su_, a_))))
                fq.sort(key=lambda fd: fd[0])
            pending = nxt
        pump((99, 99))    # drain remaining fillers (last segment's finalize)

    nc.compile()
    return nc


def _get_nc():
    if "nc" not in _CACHE:
        _CACHE["nc"] = _build()
    return _CACHE["nc"]


def _run(inputs, trace=False, tmpdir=None):
    import ml_dtypes
    from concourse.bass_utils import run_bass_kernel_spmd

    nc = _get_nc()
    q, k, v = inputs["q"], inputs["k"], inputs["v"]
    wq, wk, wv = inputs["wq"], inputs["wk"], inputs["wv"]
    bq, bk, bv = inputs["bq"], inputs["bk"], inputs["bv"]

    def f32(a):
        return np.ascontiguousarray(np.asarray(a), dtype=np.float32)

    def bf16w(a):
        return np.ascontiguousarray(
            np.asarray(a, dtype=np.float32).astype(ml_dtypes.bfloat16))

    def bf16_t(a):
        # pre-cast to the kernel's bf16 compute precision and pre-transpose
        # to the [H, S] layout its SBUF tiles use
        return np.ascontiguousarray(
            np.asarray(a, dtype=np.float32).astype(ml_dtypes.bfloat16).T)

    in_maps = []
    for c in range(CORES):
        b, g = divmod(c, CORES // B)
        sel = slice(GROUP_COLS * g, GROUP_COLS * g + GROUP_COLS)
        in_maps.append({
            "q": bf16_t(q[b]), "k": bf16_t(k[b]), "v": bf16_t(v[b]),
            "wq": bf16w(wq[:, sel]), "wk": bf16w(wk[:, sel]),
            "wv": bf16w(wv[:, sel]),
            "bq": f32(bq[sel]).reshape(GROUP_COLS, 1),
            "bk": f32(bk[sel]).reshape(GROUP_COLS, 1),
            "bv": f32(bv[sel]).reshape(GROUP_COLS, 1),
        })

    res = run_bass_kernel_spmd(nc, in_maps, list(range(CORES)),
                               trace=trace, tmpdir=tmpdir)
    out = np.empty((B, S, H), dtype=np.float32)
    for c in range(CORES):
        b, g = divmod(c, CORES // B)
        out[b, :, GROUP_COLS * g:GROUP_COLS * g + GROUP_COLS] = \
            res.results[c]["out"]
    return out, res


def kernel(**inputs):
    out, _ = _run(inputs, trace=False)
    return out


# revision 35
# speedup vs baseline: 1.2398x; 1.0444x over previous
"""Multi-head attention (B=2, S=2048, H=1024, 16 heads x 64) on 8 trn2 cores.

Sharding: data-parallel over batch (2) x tensor-parallel over heads (4 groups
of 4 heads). Core c handles batch c//4, head-group c%4 (wq/wk/wv columns
[256*g, 256*g+256)). Host slices inputs per core (shipping q/k/v pre-cast to
bf16 - the kernel's chosen compute precision - and pre-transposed to the
[H, S] layout the SBUF tiles use) and concatenates the per-core head-slice
outputs.

Per-core schedule (bf16 matmul operands, fp32 PSUM accumulation):
  ACT (exp over the 4*S*S scores) is the pacing engine at ~18.3us/segment;
  the PE runs scores (row-packed K=64 head pairs, concurrent via
  tile_position (0,0)/(64,0)), PV (65-wide stationary [V|ones] so the
  softmax denominator rides along), projections and V transposes at
  ~17us/segment underneath it.

  - prefix: one 3D-AP DMA per (tensor, 512-col chunk) (18 input triggers
    total; ~600ns/trigger on the sync/scalar queues is why they are
    batched), PE warm-up matmuls during the DMA wait (HAM un-throttle),
    then just enough projections (k nt0-1, q nt0, v nt0-1) to start the
    exp stream.
  - steady state: EGRP=2 score units share a [128,1024] PSUM tile per exp
    call; remaining projection work drip-feeds into PE slack via deadline
    fillers.
  - finalize: PSUM [65,512] out'^T tiles are copied to bf16 SBUF and
    transposed by the DMA xbar ([80,128] chunks -> [128,80]) instead of
    the PE; reciprocal of the denominator column + per-row scale write the
    staged [q,256] f32 output tiles, DMA'd out per q-tile.

The softmax mask of the reference is a mathematical no-op (it broadcasts
over the key axis, shifting every logit of a row equally), so it is ignored.
"""

import numpy as np

B, S, H = 2, 2048, 1024
NH, D = 16, 64            # heads, head_dim
CORES = 8
GROUP_COLS = 256          # 4 heads per core
SCALE = 1.0 / 32.0        # 1/sqrt(H)
EGRP = 2                  # score units (512 q cols) per exp batch

_CACHE = {}


def _build():
    import concourse.bacc as bacc
    import concourse.tile as tile
    import concourse.mybir as mybir
    from concourse.masks import make_identity
    from contextlib import ExitStack

    F32 = mybir.dt.float32
    BF16 = mybir.dt.bfloat16
    FP8 = mybir.dt.float8e4
    DR = mybir.MatmulPerfMode.DoubleRow
    EXP = mybir.ActivationFunctionType.Exp

    nc = bacc.Bacc("TRN2", target_bir_lowering=False, debug=False,
                   num_devices=CORES)

    NS = S // 128          # 16 key tiles
    NK = H // 128          # 8 contraction tiles over H
    NP = NK // 2           # fp8 DoubleRow contraction-pair tiles
    NQ = S // 512          # 4 q-tiles of 512
    NM = 2                 # head-pairs per core

    # k (and wk) arrive fp8e4 in DoubleRow-packed layout
    # [128, pair, parity, cols]; q/v stay bf16 (k-only fp8 keeps the
    # logit-noise inside the rel-err budget).
    q_d = nc.dram_tensor("q", [H, S], BF16, kind="ExternalInput").ap()
    k_d = nc.dram_tensor("k", [128, NP, 2, S], FP8, kind="ExternalInput").ap()
    v_d = nc.dram_tensor("v", [H, S], BF16, kind="ExternalInput").ap()
    w_d = {"q": nc.dram_tensor("wq", [H, GROUP_COLS], BF16,
                               kind="ExternalInput").ap(),
           "k": nc.dram_tensor("wk", [128, NP, 2, GROUP_COLS], FP8,
                               kind="ExternalInput").ap(),
           "v": nc.dram_tensor("wv", [H, GROUP_COLS], BF16,
                               kind="ExternalInput").ap()}
    b_d = {x: nc.dram_tensor("b" + x, [GROUP_COLS, 1], F32,
                             kind="ExternalInput").ap() for x in "qkv"}
    out_d = nc.dram_tensor("out", [S, GROUP_COLS], F32,
                           kind="ExternalOutput").ap()
    x_d = {"q": q_d, "k": k_d, "v": v_d}

    with tile.TileContext(nc) as tc, ExitStack() as es:
        const = es.enter_context(tc.tile_pool(name="const", bufs=1))
        wpool = es.enter_context(tc.tile_pool(name="w", bufs=1))
        xT = es.enter_context(tc.tile_pool(name="xT", bufs=1))
        proj = es.enter_context(tc.tile_pool(name="proj", bufs=1))
        vchunkp = es.enter_context(tc.tile_pool(name="vchunk", bufs=2))
        vhp = es.enter_context(tc.tile_pool(name="vh", bufs=1))
        pexpp = es.enter_context(tc.tile_pool(name="pexp", bufs=8))
        sbap = es.enter_context(tc.tile_pool(name="sba", bufs=4))
        tsbp = es.enter_context(tc.tile_pool(name="tsb", bufs=8))
        stagep = es.enter_context(tc.tile_pool(name="stage", bufs=16))
        recp = es.enter_context(tc.tile_pool(name="rec", bufs=8))
        # PSUM: sc = [128,1024] x2 = 4 banks; pa (proj acc / V transposes /
        # warmup) = 2 banks; pva/pvb = 2 banks.
        ps_sc = es.enter_context(tc.tile_pool(name="ps_sc", bufs=2, space="PSUM"))
        ps_pa = es.enter_context(tc.tile_pool(name="ps_pa", bufs=2, space="PSUM"))
        ps_pv = es.enter_context(tc.tile_pool(name="ps_pv", bufs=1, space="PSUM"))

        ident = const.tile([128, 128], F32, tag="ident")
        make_identity(nc, ident[:])
        identb = const.tile([128, 128], BF16, tag="identb")
        make_identity(nc, identb[:])

        # ---- PE warm-up: keep the array busy during the DMA prefix so the
        # HAM clock-gate releases (K=8/8) before real projections start.
        warm = const.tile([128, 512], BF16, tag="warm")
        nc.vector.memset(warm[:], 0.0)
        wps = ps_pa.tile([128, 512], F32, tag="pa", name="warmps")
        for _ in range(12):
            nc.tensor.matmul(wps[:], warm[:, 0:128], warm[:],
                             start=True, stop=True)

        # DMA priority order: everything the first projections need goes
        # first, split across many queues (a single-queue DMA moves only
        # ~75GB/s, and later DMAs on a queue wait behind earlier ones).
        engs = (nc.sync, nc.scalar)
        # k weights+inputs: fp8 DR-packed [128, pair, 2, cols]; q/v bf16
        wpt = {"k": wpool.tile([128, NP, 2, GROUP_COLS], FP8, tag="wbk",
                               name="wb_k")}
        for x in "qv":
            wpt[x] = wpool.tile([128, NK, GROUP_COLS], BF16, tag=f"wb{x}",
                                name=f"wb_{x}")
        wbf = {(x, kb): wpt[x][:, kb, :] for x in "qv" for kb in range(NK)}

        xTt = {"k": xT.tile([128, NP, 2, S], FP8, tag="xtk", name="xT_k")}
        for x in "qv":
            xTt[x] = xT.tile([128, NK, S], BF16, tag=f"xt{x}", name=f"xT_{x}")

        def dma_w(x, eng, t0, t1):
            if x == "k":
                eng.dma_start(out=wpt[x][:, t0:t1, :, :],
                              in_=w_d[x][:, t0:t1, :, :])
            else:
                eng.dma_start(
                    out=wpt[x][:, 2 * t0:2 * t1, :],
                    in_=w_d[x].rearrange("(kb p) c -> p kb c", p=128)
                    [:, 2 * t0:2 * t1, :])

        def dma_x(x, nt, eng, t0=0, t1=NP):
            cols = slice(512 * nt, 512 * nt + 512)
            if x == "k":
                eng.dma_start(out=xTt[x][:, t0:t1, :, cols],
                              in_=x_d[x][:, t0:t1, :, cols])
            else:
                eng.dma_start(
                    out=xTt[x][:, 2 * t0:2 * t1, cols],
                    in_=x_d[x].rearrange("(kb p) c -> p kb c", p=128)
                    [:, 2 * t0:2 * t1, cols])

        for i in range(2):                        # wb_k
            dma_w("k", engs[i % 2], 2 * i, 2 * i + 2)
        for t in range(NP):                       # k nt0: 4 queues
            dma_x("k", 0, engs[t % 2], t, t + 1)
        for i in range(4):                        # wb_q
            dma_w("q", engs[i % 2], i, i + 1)
        for i in range(4):                        # q nt0
            dma_x("q", 0, engs[i % 2], i, i + 1)
        for i in range(4):                        # wb_v
            dma_w("v", engs[i % 2], i, i + 1)
        for i in range(4):                        # v nt0
            dma_x("v", 0, engs[i % 2], i, i + 1)
        bias_t = {}
        for x in "qkv":
            bt = const.tile([128, NM], F32, tag=f"b{x}")
            nc.sync.dma_start(
                out=bt[:], in_=b_d[x].rearrange("(m p) o -> p m o", p=128)
                .rearrange("p m o -> p (m o)"))
            for m in range(NM):
                bias_t[(x, m)] = bt[:, m:m + 1]
        for i in range(2):                        # k nt1, v nt1
            dma_x("k", 1, engs[i], 2 * i, 2 * i + 2)
        for i in range(2):
            dma_x("v", 1, engs[i], 2 * i, 2 * i + 2)
        # late chunks all on sync: a queue-slot wait on the scalar engine
        # would block the exp stream behind it
        dma_x("k", 2, nc.sync)
        dma_x("v", 2, nc.sync)
        dma_x("k", 3, nc.sync)
        dma_x("v", 3, nc.sync)
        dma_x("q", 1, nc.sync)
        dma_x("q", 2, nc.sync)
        dma_x("q", 3, nc.sync)



        # persistent projection outputs
        QT = [proj.tile([128, S], BF16, tag=f"qt{m}", name=f"QT{m}")
              for m in range(NM)]
        KT = [proj.tile([128, S], BF16, tag=f"kt{m}", name=f"KT{m}")
              for m in range(NM)]
        VH = [[vhp.tile([128, 129], BF16, tag=f"vh{m}_{s}", name=f"VH{m}_{s}")
               for s in range(NS)] for m in range(NM)]
        for m in range(NM):
            for s in range(NS):
                nc.vector.memset(VH[m][s][:, 64:65], 1.0)

        def proj_qk_nt(x, m, nt):
            acc = ps_pa.tile([128, 512], F32, tag="pa", name="acc")
            if x == "k":
                # fp8 DoubleRow: contraction pairs (128 part x 2) per MM
                for t in range(NP):
                    nc.tensor.matmul(
                        acc[:], wpt["k"][:, t, :, 128 * m:128 * m + 128],
                        xTt["k"][:, t, :, 512 * nt:512 * nt + 512],
                        start=(t == 0), stop=(t == NP - 1), perf_mode=DR)
            else:
                for kb in range(NK):
                    nc.tensor.matmul(
                        acc[:], wbf[("q", kb)][:, 128 * m:128 * m + 128],
                        xTt["q"][:, kb, 512 * nt:512 * nt + 512],
                        start=(kb == 0), stop=(kb == NK - 1))
            dst = (QT if x == "q" else KT)[m][:, 512 * nt:512 * nt + 512]
            nc.vector.tensor_scalar_add(dst, acc[:], bias_t[(x, m)])

        def proj_v_nt(m, nt):
            acc = ps_pa.tile([128, 512], F32, tag="pa", name="acc")
            for kb in range(NK):
                nc.tensor.matmul(
                    acc[:], wbf[("v", kb)][:, 128 * m:128 * m + 128],
                    xTt["v"][:, kb, 512 * nt:512 * nt + 512],
                    start=(kb == 0), stop=(kb == NK - 1))
            vchunk = vchunkp.tile([128, 512], BF16, tag="vchunk", name="vchunk")
            nc.vector.tensor_scalar_add(vchunk[:], acc[:], bias_t[("v", m)])
            for i in range(4):
                s = 4 * nt + i
                trp = ps_pa.tile([128, 128], BF16, tag="pa", name="trv")
                nc.tensor.transpose(trp[:], vchunk[:, 128 * i:128 * i + 128],
                                    identb[:])
                vt = VH[m][s]
                nc.vector.tensor_copy(vt[:, 0:64], trp[:, 0:64])
                nc.vector.tensor_copy(vt[:, 65:129], trp[:, 64:128])

        stages = {}
        for qt in range(NQ):
            stages[qt] = [stagep.tile([128, GROUP_COLS], F32, tag="stage",
                                      name=f"stage{qt}_{i}") for i in range(4)]

        # ---- pre-work: just enough to start the exp stream ----
        proj_qk_nt("k", 0, 0)
        proj_qk_nt("q", 0, 0)
        proj_v_nt(0, 0)
        proj_qk_nt("k", 0, 1)
        proj_v_nt(0, 1)

        # ---- attention pipeline with deadline-driven PE fillers ----
        units = [(kt, a) for kt in range(NS) for a in (0, 1)]
        grps = [units[i:i + EGRP] for i in range(0, len(units), EGRP)]
        NG = len(grps)

        # m-major segment order
        segs = [{"qt": qt, "m": m, "pva": None, "pvb": None, "idx": 4 * m + qt}
                for m in range(NM) for qt in range(NQ)]

        # fillers: (deadline (seg_idx, gi) = emit before that slot's pv, fn)
        fq = [
            ((0, 5), lambda: proj_qk_nt("k", 0, 2)),
            ((0, 7), lambda: proj_v_nt(0, 2)),
            ((0, 9), lambda: proj_qk_nt("k", 0, 3)),
            ((0, 11), lambda: proj_v_nt(0, 3)),
            ((0, 14), lambda: proj_qk_nt("q", 0, 1)),    # QT[0] for seg 1
            ((1, 3), lambda: proj_qk_nt("k", 1, 0)),
            ((1, 7), lambda: proj_qk_nt("k", 1, 1)),
            ((1, 14), lambda: proj_qk_nt("q", 0, 2)),    # QT[0] for seg 2
            ((2, 3), lambda: proj_qk_nt("k", 1, 2)),
            ((2, 7), lambda: proj_qk_nt("k", 1, 3)),
            ((2, 14), lambda: proj_qk_nt("q", 0, 3)),    # QT[0] for seg 3
            ((3, 3), lambda: proj_v_nt(1, 0)),           # VH[1][kt 0..3]
            ((3, 14), lambda: proj_qk_nt("q", 1, 0)),    # QT[1] for seg 4
            ((4, 3), lambda: proj_v_nt(1, 1)),
            ((4, 7), lambda: proj_v_nt(1, 2)),
            ((4, 11), lambda: proj_v_nt(1, 3)),
            ((4, 14), lambda: proj_qk_nt("q", 1, 1)),    # QT[1] for seg 5
            ((5, 14), lambda: proj_qk_nt("q", 1, 2)),
            ((6, 14), lambda: proj_qk_nt("q", 1, 3)),
        ]
        fq.sort(key=lambda fd: fd[0])

        def pump(upto):
            while fq and fq[0][0] <= upto:
                fq.pop(0)[1]()

        def emit_scores(seg, g):
            qt, m = seg["qt"], seg["m"]
            stt = ps_sc.tile([128, 1024], F32, tag="sc", name="stt")
            for u, (kt, a) in enumerate(g):
                p0 = 64 * a
                nc.tensor.matmul(
                    stt[:, 512 * u:512 * u + 512],
                    KT[m][p0:p0 + 64, 128 * kt:128 * kt + 128],
                    QT[m][p0:p0 + 64, 512 * qt:512 * qt + 512],
                    start=True, stop=True, tile_position=(p0, 0))
            pe = pexpp.tile([128, 1024], BF16, tag="pexp", name="pexp")
            n = 512 * len(g)
            nc.scalar.activation(pe[:, 0:n], stt[:, 0:n], EXP, scale=SCALE)
            return pe

        def emit_pv(seg, g, pe):
            m = seg["m"]
            if seg["pva"] is None:
                seg["pva"] = ps_pv.tile([65, 512], F32, tag="pva", name="pva")
                seg["pvb"] = ps_pv.tile([65, 512], F32, tag="pvb", name="pvb")
            for u, (kt, a) in enumerate(g):
                pv = seg["pva"] if a == 0 else seg["pvb"]
                lo = 64 * a
                nc.tensor.matmul(pv[:], VH[m][kt][:, lo:lo + 65],
                                 pe[:, 512 * u:512 * u + 512],
                                 start=(kt == 0), stop=(kt == NS - 1))

        # finalize: the pva/pvb->SBUF copies run immediately (freeing the
        # PSUM banks); the transpose/divide/stage steps become fillers
        # spread over the following segment's PE slack.
        # pva cols were [A(64)|ones] -> denom at row 64, dims at 0:64;
        # pvb cols were [ones|B(64)] -> denom at row 0, dims at 1:65.
        stage_done = {}

        def fin_item(seg, sb, sub, a):
            qt, m = seg["qt"], seg["m"]
            stage = stages[qt]
            trp = ps_pa.tile([128, 128], F32, tag="pa", name="trf")
            nc.tensor.transpose(trp[:, 0:65],
                                sb[0:65, 128 * sub:128 * sub + 128],
                                ident[0:65, 0:65])
            # one fast copy releases the PSUM slot; divide from SBUF
            tsb = tsbp.tile([128, 65], F32, tag="tsb", name="tsb")
            nc.vector.tensor_copy(tsb[:], trp[:, 0:65])
            r = recp.tile([128, 1], F32, tag="rec", name="r")
            dcol = 64 if a == 0 else 0
            lo, hi = (0, 64) if a == 0 else (1, 65)
            nc.vector.reciprocal(r[:], tsb[:, dcol:dcol + 1])
            nc.vector.tensor_scalar_mul(
                stage[sub][:, 128 * m + 64 * a:128 * m + 64 * a + 64],
                tsb[:, lo:hi], r[:, 0:1])
            # each [q-sub, 256] stage tile DMAs out as soon as its 4
            # (head-pair, head) slices are in, spreading the output DMAs
            k2 = (qt, sub)
            stage_done[k2] = stage_done.get(k2, 0) + 1
            if stage_done[k2] == 4:
                nc.sync.dma_start(
                    out=out_d[512 * qt + 128 * sub:512 * qt + 128 * sub + 128, :],
                    in_=stage[sub][:])

        flat = [(seg, gi) for seg in segs for gi in range(NG)]
        pending = emit_scores(flat[0][0], grps[flat[0][1]])
        for j, (seg, gi) in enumerate(flat):
            nxt = None
            if j + 1 < len(flat):
                nseg, ngi = flat[j + 1]
                nxt = emit_scores(nseg, grps[ngi])
            pump((seg["idx"], gi))
            emit_pv(seg, grps[gi], pending)
            pending = nxt
            if gi == NG - 1:
                nidx = seg["idx"] + 1
                for a in (0, 1):
                    pv = seg["pva"] if a == 0 else seg["pvb"]
                    sb = sbap.tile([65, 512], F32, tag="sba", name="sb")
                    nc.vector.tensor_copy(sb[:], pv[:])
                    for sub in range(4):
                        fq.append(((nidx, 1 + 2 * sub + a),
                                   (lambda s_=seg, sb_=sb, su_=sub, a_=a:
                                    fin_item(s_, sb_, su_, a_))))
                fq.sort(key=lambda fd: fd[0])
        pump((99, 99))    # drain remaining fillers (last segment's finalize)

    nc.compile()
    return nc


def _get_nc():
    if "nc" not in _CACHE:
        _CACHE["nc"] = _build()
    return _CACHE["nc"]


def _in_maps(inputs):
    import ml_dtypes

    q, k, v = inputs["q"], inputs["k"], inputs["v"]
    wq, wk, wv = inputs["wq"], inputs["wk"], inputs["wv"]
    bq, bk, bv = inputs["bq"], inputs["bk"], inputs["bv"]
    NP = H // 256

    def f32(a):
        return np.ascontiguousarray(np.asarray(a), dtype=np.float32)

    def bf16w(a):
        return np.ascontiguousarray(
            np.asarray(a, dtype=np.float32).astype(ml_dtypes.bfloat16))

    def bf16_t(a):
        # pre-cast to the kernel's bf16 compute precision and pre-transpose
        # to the [H, S] layout its SBUF tiles use
        return np.ascontiguousarray(
            np.asarray(a, dtype=np.float32).astype(ml_dtypes.bfloat16).T)

    def fp8_pack(hs):
        # [H, cols] -> DoubleRow layout [128, pair, parity, cols] fp8e4
        a = np.asarray(hs, dtype=np.float32).astype(ml_dtypes.float8_e4m3fn)
        return np.ascontiguousarray(
            a.reshape(NP, 2, 128, a.shape[1]).transpose(2, 0, 1, 3))

    in_maps = []
    for c in range(CORES):
        b, g = divmod(c, CORES // B)
        sel = slice(GROUP_COLS * g, GROUP_COLS * g + GROUP_COLS)
        in_maps.append({
            "q": bf16_t(q[b]), "k": fp8_pack(np.asarray(k[b]).T),
            "v": bf16_t(v[b]),
            "wq": bf16w(wq[:, sel]), "wk": fp8_pack(wk[:, sel]),
            "wv": bf16w(wv[:, sel]),
            "bq": f32(bq[sel]).reshape(GROUP_COLS, 1),
            "bk": f32(bk[sel]).reshape(GROUP_COLS, 1),
            "bv": f32(bv[sel]).reshape(GROUP_COLS, 1),
        })
    return in_maps


def _run(inputs, trace=False, tmpdir=None):
    from concourse.bass_utils import run_bass_kernel_spmd

    nc = _get_nc()
    in_maps = _in_maps(inputs)
    res = run_bass_kernel_spmd(nc, in_maps, list(range(CORES)),
                               trace=trace, tmpdir=tmpdir)
    out = np.empty((B, S, H), dtype=np.float32)
    for c in range(CORES):
        b, g = divmod(c, CORES // B)
        out[b, :, GROUP_COLS * g:GROUP_COLS * g + GROUP_COLS] = \
            res.results[c]["out"]
    return out, res


def kernel(**inputs):
    out, _ = _run(inputs, trace=False)
    return out
